# revision 18
# baseline (speedup 1.0000x reference)
"""DLinear fused kernel for 8 TRN2 NeuronCores.

Math: the whole module is linear in x.
  trend = x @ A^T (A = edge-padded moving-average matrix, window 25)
  out[b,n,:] = sum_c wf_c * ( x[b,c,n,:] @ (Ws + (Wt-Ws)@A)^T ) + bias
  bias = sum(wf) * (bs + bt) + bf

Host precomputes the tiny effective weight Weff = Ws + (Wt-Ws)@A in f64
(weights only) and folds the per-channel scalars wf_c into the f32->bf16
input cast. Device per core (8 batches):
  - channel combine xc = (xa + xb) + xk: two plain DVE tensor_tensor adds
    (bf16 2x perf mode, ~0.69us each; scalar_tensor_tensor would be 1x).
  - matmul in (bb, nt) groups of 3 PSUM banks (8-buf pool): for k(4) x
    pc(3), MM [112p x 512bn] accumulated over the 4 l-chunks.
  - an ~11-matmul zero warmup stream at the head keeps the PE HAM
    un-throttled (2.4 GHz) until the first data-dependent matmul; cold
    (1.2 GHz) matmuls otherwise lag the DMA pace and cascade into an
    input-stream/output collision at the tail.
  - PSUM drains split across engines: nt=0 on ScalarE (activation with
    fused bias), nt=1 on DVE (tensor_scalar per-partition bias add).
DMA: 16 x 768KB input transfers (6KB-contiguous rows) ALONE on the Sync
HWDGE ring; weights/bias via the Scalar HWDGE ring (early, fast);
outputs via the GpSimd SWDGE ring so stores never queue behind loads.
bb2's outputs are explicitly held until the last input transfer so the
store burst cannot starve the final input chunks (which otherwise
trickle for ~6us while the PE waits).
"""

import numpy as np
import ml_dtypes

import concourse.bacc as bacc
import concourse.mybir as mybir
import concourse.tile as tile
from concourse.bass import _add_dep_helper
from concourse.bass_utils import run_bass_kernel_spmd

N_CORES = 8
B, C, N, L, P = 64, 3, 512, 512, 336
KERNEL_W, PAD = 25, 12
BPC = B // N_CORES          # batches per core = 8
BN = BPC * N                # rows per core = 4096
BB, BNB = 4, 1024           # bn blocks per core, rows per block
LC = 4                      # l chunks of 128
PC, PCW = 3, 112            # p chunks x width (3*112 = 336)
NT, NTW = 2, 512            # bn tiles per block x width
N_WARM = 11                 # zero matmuls to pre-warm the PE HAM

BF16 = mybir.dt.bfloat16
F32 = mybir.dt.float32

LAST_RESULT = None
_CACHE = {}


def _movavg_matrix():
    A = np.zeros((L, L), np.float64)
    for lp in range(L):
        for kk in range(lp - PAD, lp + PAD + 1):
            A[lp, min(max(kk, 0), L - 1)] += 1.0 / KERNEL_W
    return A


def _build():
    nc = bacc.Bacc("TRN2", target_bir_lowering=False, debug=False)
    x_d = nc.dram_tensor("x", (BB, LC, 128, C * BNB), BF16,
                         kind="ExternalInput")
    w_d = nc.dram_tensor("w", (LC, 128, P), BF16, kind="ExternalInput")
    b_d = nc.dram_tensor("bias", (PCW, PC), F32, kind="ExternalInput")
    o_d = nc.dram_tensor("o", (BB, PC, PCW, BNB), BF16,
                         kind="ExternalOutput")

    with tile.TileContext(nc) as tc:
        with (
            tc.tile_pool(name="const", bufs=1) as constp,
            tc.tile_pool(name="xin", bufs=16) as xinp,
            tc.tile_pool(name="tmid", bufs=4) as tp,
            tc.tile_pool(name="xcp", bufs=8) as xcp,
            tc.tile_pool(name="ps", bufs=8, space="PSUM") as psp,
            tc.tile_pool(name="ostage", bufs=3) as osp,
        ):
            # PE warmup: zero matmuls fill the HAM busy-window until the
            # first data-dependent matmul so real matmuls run at 2.4 GHz.
            zt = constp.tile([128, NTW + PCW], BF16, tag="zt", name="zt")
            nc.vector.memset(zt[:], 0.0)
            wup = psp.tile([PCW, NTW], F32, tag="ps", name="warmps")
            for i in range(N_WARM):
                nc.tensor.matmul(wup[:], zt[:, NTW:NTW + PCW], zt[:, 0:NTW],
                                 start=True, stop=True)

            wts = []
            for k in range(LC):
                wt = constp.tile([128, P], BF16, tag=f"w{k}", name=f"w{k}")
                nc.scalar.dma_start(wt[:], w_d[k])
                wts.append(wt)
            btile = constp.tile([PCW, PC], F32, tag="bias", name="bias")
            nc.scalar.dma_start(btile[:], b_d[:])

            last_x_dma = None
            out_dmas_bb2 = []
            for bb in range(BB):
                xcs = []
                for lc in range(LC):
                    xf = xinp.tile([128, C * BNB], BF16, tag="xin",
                                   name=f"x{bb}_{lc}")
                    last_x_dma = nc.sync.dma_start(xf[:], x_d[bb, lc])
                    xa = xf[:, 0:BNB]
                    xb = xf[:, BNB:2 * BNB]
                    xk = xf[:, 2 * BNB:3 * BNB]
                    t = tp.tile([128, BNB], BF16, tag="t",
                                name=f"t{lc}_{bb}")
                    nc.vector.tensor_add(t[:], xa, xb)
                    xc = xcp.tile([128, BNB], BF16, tag="xc",
                                  name=f"xc{lc}_{bb}")
                    nc.vector.tensor_add(xc[:], t[:], xk)
                    xcs.append(xc)

                osts = []
                for pc in range(PC):
                    ost = osp.tile([PCW, BNB], BF16, tag=f"ost{pc}",
                                   name=f"ost{bb}_{pc}")
                    osts.append(ost)

                # (bb, nt) groups of 3 PSUM banks; k-inner so each bank is
                # held for only 12 matmuls before its drain frees it.
                for nt in range(NT):
                    pss = [
                        psp.tile([PCW, NTW], F32, tag="ps",
                                 name=f"ps{bb}_{nt}_{pc}")
                        for pc in range(PC)
                    ]
                    for k in range(LC):
                        for pc in range(PC):
                            nc.tensor.matmul(
                                pss[pc][:],
                                wts[k][:, pc * PCW:(pc + 1) * PCW],
                                xcs[k][:, nt * NTW:(nt + 1) * NTW],
                                start=(k == 0),
                                stop=(k == LC - 1),
                            )
                    for pc in range(PC):
                        dst = osts[pc][:, nt * NTW:(nt + 1) * NTW]
                        if nt == 0:
                            nc.scalar.activation(
                                dst, pss[pc][:],
                                mybir.ActivationFunctionType.Identity,
                                bias=btile[:, pc:pc + 1],
                            )
                        else:
                            nc.vector.tensor_scalar(
                                dst, pss[pc][:], btile[:, pc:pc + 1],
                                None, mybir.AluOpType.add,
                            )
                for pc in range(PC):
                    od = nc.gpsimd.dma_start(o_d[bb, pc], osts[pc][:])
                    if bb == 2:
                        out_dmas_bb2.append(od)

            # bb2's stores land right when the final input chunks stream;
            # hold them until the input FIFO is fully drained.
            for od in out_dmas_bb2:
                _add_dep_helper(od.ins, last_x_dma.ins, sync=True,
                                reason="defer bb2 stores past input stream")

    nc.compile()
    return nc


def kernel(x, Ws, bs, Wt, bt, Wf, bf):
    global LAST_RESULT
    # ---- host-side weight folding (f64, weights only) ----
    A = _movavg_matrix()
    Weff = Ws.astype(np.float64) + (Wt.astype(np.float64) - Ws.astype(np.float64)) @ A
    WT = np.ascontiguousarray(Weff.T).reshape(LC, 128, P).astype(ml_dtypes.bfloat16)
    wf = Wf[0].astype(np.float64)                      # (3,)
    bias = wf.sum() * (bs.astype(np.float64) + bt.astype(np.float64)) + float(bf[0])
    bias_r = np.ascontiguousarray(bias.astype(np.float32).reshape(PC, PCW).T)

    # ---- build / compile (cached; no runtime scalars baked in) ----
    if "nc" not in _CACHE:
        _CACHE["nc"] = _build()
    nc = _CACHE["nc"]

    # ---- host-side sharding / layout; wf folded into the bf16 cast ----
    xs = x * Wf[0][None, :, None, None]                # (64,3,512,512) f32
    xb16 = xs.astype(ml_dtypes.bfloat16)
    xr = xb16.reshape(N_CORES, BPC, C, N, L)
    xr = xr.transpose(0, 2, 4, 1, 3)                   # [core, c, l, bl, n]
    xr = xr.reshape(N_CORES, C, LC, 128, BB, BNB)
    xr = xr.transpose(0, 4, 2, 3, 1, 5)                # [core, bb, lc, 128, c, bn]
    xr = xr.reshape(N_CORES, BB, LC, 128, C * BNB)

    in_maps = []
    for i in range(N_CORES):
        in_maps.append({
            "x": np.ascontiguousarray(xr[i]),
            "w": WT,
            "bias": bias_r,
        })

    res = run_bass_kernel_spmd(nc, in_maps, core_ids=list(range(N_CORES)))
    LAST_RESULT = res

    # ---- gather / unshard ----
    outs = []
    for i in range(N_CORES):
        o = res.results[i]["o"].astype(np.float32)     # (4, 3, 112, 1024)
        o = o.transpose(0, 3, 1, 2).reshape(BPC, N, P)
        outs.append(o)
    out = np.stack(outs).reshape(B, N, P)[:, None]     # (64, 1, 512, 336)
    return out.astype(np.float32)


# revision 19
# speedup vs baseline: 1.0965x; 1.0965x over previous
"""DLinear fused kernel for 8 TRN2 NeuronCores.

Math: the whole module is linear in x.
  trend = x @ A^T (A = edge-padded moving-average matrix, window 25)
  out[b,n,:] = sum_c wf_c * ( x[b,c,n,:] @ (Ws + (Wt-Ws)@A)^T ) + bias
  bias = sum(wf) * (bs + bt) + bf

Host precomputes the tiny effective weight Weff = Ws + (Wt-Ws)@A in f64
(weights only) and folds the per-channel scalars wf_c into the f32->bf16
input cast. Device per core (8 batches):
  - channel combine xc = (xa + xb) + xk: two plain DVE tensor_tensor adds
    (bf16 2x perf mode, ~0.69us each; scalar_tensor_tensor would be 1x).
  - matmul in (bb, nt) groups of 3 PSUM banks (8-buf pool): for k(4) x
    pc(3), MM [112p x 512bn] accumulated over the 4 l-chunks.
  - an ~11-matmul zero warmup stream at the head keeps the PE HAM
    un-throttled (2.4 GHz) until the first data-dependent matmul; cold
    (1.2 GHz) matmuls otherwise lag the DMA pace and cascade into an
    input-stream/output collision at the tail.
  - PSUM drains split across engines: nt=0 on ScalarE (activation with
    fused bias), nt=1 on DVE (tensor_scalar per-partition bias add).
DMA: 16 x 768KB input transfers (6KB-contiguous rows) ALONE on the Sync
HWDGE ring; weights/bias via the Scalar HWDGE ring (early, fast);
outputs via the GpSimd SWDGE ring so stores never queue behind loads,
one 688KB store per bb ([112, 3*1024] staging tile, 6KB descriptors),
ALL held until the last input transfer so store traffic can never starve
the input stream (which otherwise dips to ~280 GB/s and triggers HAM
re-throttles).
"""

import numpy as np
import ml_dtypes

import concourse.bacc as bacc
import concourse.mybir as mybir
import concourse.tile as tile
from concourse.bass import _add_dep_helper
from concourse.bass_utils import run_bass_kernel_spmd

N_CORES = 8
B, C, N, L, P = 64, 3, 512, 512, 336
KERNEL_W, PAD = 25, 12
BPC = B // N_CORES          # batches per core = 8
BN = BPC * N                # rows per core = 4096
BB, BNB = 4, 1024           # bn blocks per core, rows per block
LC = 4                      # l chunks of 128
PC, PCW = 3, 112            # p chunks x width (3*112 = 336)
NT, NTW = 2, 512            # bn tiles per block x width
N_WARM = 11                 # zero matmuls to pre-warm the PE HAM

BF16 = mybir.dt.bfloat16
F32 = mybir.dt.float32

LAST_RESULT = None
_CACHE = {}


def _movavg_matrix():
    A = np.zeros((L, L), np.float64)
    for lp in range(L):
        for kk in range(lp - PAD, lp + PAD + 1):
            A[lp, min(max(kk, 0), L - 1)] += 1.0 / KERNEL_W
    return A


def _build():
    nc = bacc.Bacc("TRN2", target_bir_lowering=False, debug=False)
    x_d = nc.dram_tensor("x", (BB, LC, 128, C * BNB), BF16,
                         kind="ExternalInput")
    w_d = nc.dram_tensor("w", (LC, 128, P), BF16, kind="ExternalInput")
    b_d = nc.dram_tensor("bias", (PCW, PC), F32, kind="ExternalInput")
    o_d = nc.dram_tensor("o", (BB, PCW, PC * BNB), BF16,
                         kind="ExternalOutput")

    with tile.TileContext(nc) as tc:
        with (
            tc.tile_pool(name="const", bufs=1) as constp,
            tc.tile_pool(name="xin", bufs=16) as xinp,
            tc.tile_pool(name="tmid", bufs=4) as tp,
            tc.tile_pool(name="xcp", bufs=8) as xcp,
            tc.tile_pool(name="ps", bufs=8, space="PSUM") as psp,
            tc.tile_pool(name="ostage", bufs=4) as osp,
        ):
            # PE warmup: zero matmuls fill the HAM busy-window until the
            # first data-dependent matmul so real matmuls run at 2.4 GHz.
            zt = constp.tile([128, NTW + PCW], BF16, tag="zt", name="zt")
            nc.vector.memset(zt[:], 0.0)
            wup = psp.tile([PCW, NTW], F32, tag="ps", name="warmps")
            for i in range(N_WARM):
                nc.tensor.matmul(wup[:], zt[:, NTW:NTW + PCW], zt[:, 0:NTW],
                                 start=True, stop=True)

            wts = []
            for k in range(LC):
                wt = constp.tile([128, P], BF16, tag=f"w{k}", name=f"w{k}")
                nc.scalar.dma_start(wt[:], w_d[k])
                wts.append(wt)
            btile = constp.tile([PCW, PC], F32, tag="bias", name="bias")
            nc.scalar.dma_start(btile[:], b_d[:])

            last_x_dma = None
            deferred_outs = []
            for bb in range(BB):
                xcs = []
                for lc in range(LC):
                    xf = xinp.tile([128, C * BNB], BF16, tag="xin",
                                   name=f"x{bb}_{lc}")
                    last_x_dma = nc.sync.dma_start(xf[:], x_d[bb, lc])
                    xa = xf[:, 0:BNB]
                    xb = xf[:, BNB:2 * BNB]
                    xk = xf[:, 2 * BNB:3 * BNB]
                    t = tp.tile([128, BNB], BF16, tag="t",
                                name=f"t{lc}_{bb}")
                    nc.vector.tensor_add(t[:], xa, xb)
                    xc = xcp.tile([128, BNB], BF16, tag="xc",
                                  name=f"xc{lc}_{bb}")
                    nc.vector.tensor_add(xc[:], t[:], xk)
                    xcs.append(xc)

                ost = osp.tile([PCW, PC * BNB], BF16, tag="ost",
                               name=f"ost{bb}")

                # (bb, nt) groups of 3 PSUM banks; k-inner so each bank is
                # held for only 12 matmuls before its drain frees it.
                for nt in range(NT):
                    pss = [
                        psp.tile([PCW, NTW], F32, tag="ps",
                                 name=f"ps{bb}_{nt}_{pc}")
                        for pc in range(PC)
                    ]
                    for k in range(LC):
                        for pc in range(PC):
                            nc.tensor.matmul(
                                pss[pc][:],
                                wts[k][:, pc * PCW:(pc + 1) * PCW],
                                xcs[k][:, nt * NTW:(nt + 1) * NTW],
                                start=(k == 0),
                                stop=(k == LC - 1),
                            )
                    for pc in range(PC):
                        base = pc * BNB + nt * NTW
                        dst = ost[:, base:base + NTW]
                        if nt == 0:
                            nc.scalar.activation(
                                dst, pss[pc][:],
                                mybir.ActivationFunctionType.Identity,
                                bias=btile[:, pc:pc + 1],
                            )
                        else:
                            nc.vector.tensor_scalar(
                                dst, pss[pc][:], btile[:, pc:pc + 1],
                                None, mybir.AluOpType.add,
                            )
                od = nc.gpsimd.dma_start(o_d[bb], ost[:])
                if bb < BB - 1:
                    deferred_outs.append(od)

            # Stores interleaving with the input stream knock it from
            # ~420 to ~280 GB/s, starving the PE into HAM re-throttles;
            # hold every early store until the input FIFO is drained
            # (bb3's store is naturally later).
            for od in deferred_outs:
                _add_dep_helper(od.ins, last_x_dma.ins, sync=True,
                                reason="defer stores past input stream")

    nc.compile()
    return nc


def kernel(x, Ws, bs, Wt, bt, Wf, bf):
    global LAST_RESULT
    # ---- host-side weight folding (f64, weights only) ----
    A = _movavg_matrix()
    Weff = Ws.astype(np.float64) + (Wt.astype(np.float64) - Ws.astype(np.float64)) @ A
    WT = np.ascontiguousarray(Weff.T).reshape(LC, 128, P).astype(ml_dtypes.bfloat16)
    wf = Wf[0].astype(np.float64)                      # (3,)
    bias = wf.sum() * (bs.astype(np.float64) + bt.astype(np.float64)) + float(bf[0])
    bias_r = np.ascontiguousarray(bias.astype(np.float32).reshape(PC, PCW).T)

    # ---- build / compile (cached; no runtime scalars baked in) ----
    if "nc" not in _CACHE:
        _CACHE["nc"] = _build()
    nc = _CACHE["nc"]

    # ---- host-side sharding / layout; wf folded into the bf16 cast ----
    xs = x * Wf[0][None, :, None, None]                # (64,3,512,512) f32
    xb16 = xs.astype(ml_dtypes.bfloat16)
    xr = xb16.reshape(N_CORES, BPC, C, N, L)
    xr = xr.transpose(0, 2, 4, 1, 3)                   # [core, c, l, bl, n]
    xr = xr.reshape(N_CORES, C, LC, 128, BB, BNB)
    xr = xr.transpose(0, 4, 2, 3, 1, 5)                # [core, bb, lc, 128, c, bn]
    xr = xr.reshape(N_CORES, BB, LC, 128, C * BNB)

    in_maps = []
    for i in range(N_CORES):
        in_maps.append({
            "x": np.ascontiguousarray(xr[i]),
            "w": WT,
            "bias": bias_r,
        })

    res = run_bass_kernel_spmd(nc, in_maps, core_ids=list(range(N_CORES)))
    LAST_RESULT = res

    # ---- gather / unshard ----
    outs = []
    for i in range(N_CORES):
        o = res.results[i]["o"].astype(np.float32)     # (4, 112, 3*1024)
        o = o.reshape(BB, PCW, PC, BNB)
        o = o.transpose(0, 3, 2, 1).reshape(BPC, N, P)
        outs.append(o)
    out = np.stack(outs).reshape(B, N, P)[:, None]     # (64, 1, 512, 336)
    return out.astype(np.float32)


# revision 20
# speedup vs baseline: 1.1669x; 1.0643x over previous
"""DLinear fused kernel for 8 TRN2 NeuronCores.

Math: the whole module is linear in x.
  trend = x @ A^T (A = edge-padded moving-average matrix, window 25)
  out[b,n,:] = sum_c wf_c * ( x[b,c,n,:] @ (Ws + (Wt-Ws)@A)^T ) + bias
  bias = sum(wf) * (bs + bt) + bf

Host precomputes the tiny effective weight Weff = Ws + (Wt-Ws)@A in f64
(weights only) and folds the per-channel scalars wf_c into the f32->bf16
input cast. Device per core (8 batches):
  - channel combine xc = (xa + xb) + xk: two plain DVE tensor_tensor adds
    (bf16 2x perf mode, ~0.69us each; scalar_tensor_tensor would be 1x).
  - matmul in (bb, nt) groups of 3 PSUM banks (8-buf pool): for k(4) x
    pc(3), MM [112p x 512bn] accumulated over the 4 l-chunks.
  - an ~11-matmul zero warmup stream at the head keeps the PE HAM
    un-throttled (2.4 GHz) until the first data-dependent matmul; cold
    (1.2 GHz) matmuls otherwise lag the DMA pace and cascade into an
    input-stream/output collision at the tail.
  - PSUM drains on ScalarE (activation with fused bias); only the final
    group's drains go to the DVE (tensor_scalar per-partition bias add),
    once the DVE has no more chunk adds to feed the PE.
DMA: 16 x 768KB input transfers (6KB-contiguous rows) ALONE on the Sync
HWDGE ring; weights/bias via the Scalar HWDGE ring (early, fast);
outputs via the GpSimd SWDGE ring so stores never queue behind loads,
one 688KB store per bb ([112, 3*1024] staging tile, 6KB descriptors),
ALL held until the last input transfer so store traffic can never starve
the input stream (which otherwise dips to ~280 GB/s and triggers HAM
re-throttles).
"""

import numpy as np
import ml_dtypes

import concourse.bacc as bacc
import concourse.mybir as mybir
import concourse.tile as tile
from concourse.bass import _add_dep_helper
from concourse.bass_utils import run_bass_kernel_spmd

N_CORES = 8
B, C, N, L, P = 64, 3, 512, 512, 336
KERNEL_W, PAD = 25, 12
BPC = B // N_CORES          # batches per core = 8
BN = BPC * N                # rows per core = 4096
BB, BNB = 4, 1024           # bn blocks per core, rows per block
LC = 4                      # l chunks of 128
PC, PCW = 3, 112            # p chunks x width (3*112 = 336)
NT, NTW = 2, 512            # bn tiles per block x width
N_WARM = 11                 # zero matmuls to pre-warm the PE HAM

BF16 = mybir.dt.bfloat16
F32 = mybir.dt.float32

LAST_RESULT = None
_CACHE = {}


def _movavg_matrix():
    A = np.zeros((L, L), np.float64)
    for lp in range(L):
        for kk in range(lp - PAD, lp + PAD + 1):
            A[lp, min(max(kk, 0), L - 1)] += 1.0 / KERNEL_W
    return A


def _build():
    nc = bacc.Bacc("TRN2", target_bir_lowering=False, debug=False)
    x_d = nc.dram_tensor("x", (BB, LC, 128, C * BNB), BF16,
                         kind="ExternalInput")
    w_d = nc.dram_tensor("w", (LC, 128, P), BF16, kind="ExternalInput")
    b_d = nc.dram_tensor("bias", (PCW, PC), F32, kind="ExternalInput")
    o_d = nc.dram_tensor("o", (BB, PCW, PC * BNB), BF16,
                         kind="ExternalOutput")

    with tile.TileContext(nc) as tc:
        with (
            tc.tile_pool(name="const", bufs=1) as constp,
            tc.tile_pool(name="xin", bufs=16) as xinp,
            tc.tile_pool(name="tmid", bufs=4) as tp,
            tc.tile_pool(name="xcp", bufs=8) as xcp,
            tc.tile_pool(name="ps", bufs=8, space="PSUM") as psp,
            tc.tile_pool(name="ostage", bufs=4) as osp,
        ):
            # PE warmup: zero matmuls fill the HAM busy-window until the
            # first data-dependent matmul so real matmuls run at 2.4 GHz.
            zt = constp.tile([128, NTW + PCW], BF16, tag="zt", name="zt")
            nc.vector.memset(zt[:], 0.0)
            wup = psp.tile([PCW, NTW], F32, tag="ps", name="warmps")
            for i in range(N_WARM):
                nc.tensor.matmul(wup[:], zt[:, NTW:NTW + PCW], zt[:, 0:NTW],
                                 start=True, stop=True)

            wts = []
            for k in range(LC):
                wt = constp.tile([128, P], BF16, tag=f"w{k}", name=f"w{k}")
                nc.scalar.dma_start(wt[:], w_d[k])
                wts.append(wt)
            btile = constp.tile([PCW, PC], F32, tag="bias", name="bias")
            nc.scalar.dma_start(btile[:], b_d[:])

            last_x_dma = None
            deferred_outs = []
            for bb in range(BB):
                xcs = []
                for lc in range(LC):
                    xf = xinp.tile([128, C * BNB], BF16, tag="xin",
                                   name=f"x{bb}_{lc}")
                    last_x_dma = nc.sync.dma_start(xf[:], x_d[bb, lc])
                    xa = xf[:, 0:BNB]
                    xb = xf[:, BNB:2 * BNB]
                    xk = xf[:, 2 * BNB:3 * BNB]
                    t = tp.tile([128, BNB], BF16, tag="t",
                                name=f"t{lc}_{bb}")
                    nc.vector.tensor_add(t[:], xa, xb)
                    xc = xcp.tile([128, BNB], BF16, tag="xc",
                                  name=f"xc{lc}_{bb}")
                    nc.vector.tensor_add(xc[:], t[:], xk)
                    xcs.append(xc)

                ost = osp.tile([PCW, PC * BNB], BF16, tag="ost",
                               name=f"ost{bb}")

                # (bb, nt) groups of 3 PSUM banks; k-inner so each bank is
                # held for only 12 matmuls before its drain frees it.
                for nt in range(NT):
                    pss = [
                        psp.tile([PCW, NTW], F32, tag="ps",
                                 name=f"ps{bb}_{nt}_{pc}")
                        for pc in range(PC)
                    ]
                    for k in range(LC):
                        for pc in range(PC):
                            nc.tensor.matmul(
                                pss[pc][:],
                                wts[k][:, pc * PCW:(pc + 1) * PCW],
                                xcs[k][:, nt * NTW:(nt + 1) * NTW],
                                start=(k == 0),
                                stop=(k == LC - 1),
                            )
                    for pc in range(PC):
                        base = pc * BNB + nt * NTW
                        dst = ost[:, base:base + NTW]
                        # DVE must stay clear for the chunk adds (drains
                        # queued ahead of adds in its FIFO delay xc by
                        # ~2us at every bb boundary and stall the PE into
                        # HAM re-throttle); only the final group, when
                        # all adds are done, drains on DVE.
                        if not (bb == BB - 1 and nt == 1):
                            nc.scalar.activation(
                                dst, pss[pc][:],
                                mybir.ActivationFunctionType.Identity,
                                bias=btile[:, pc:pc + 1],
                            )
                        else:
                            nc.vector.tensor_scalar(
                                dst, pss[pc][:], btile[:, pc:pc + 1],
                                None, mybir.AluOpType.add,
                            )
                od = nc.gpsimd.dma_start(o_d[bb], ost[:])
                if bb < BB - 1:
                    deferred_outs.append(od)

            # Stores interleaving with the input stream knock it from
            # ~420 to ~280 GB/s, starving the PE into HAM re-throttles;
            # hold every early store until the input FIFO is drained
            # (bb3's store is naturally later).
            for od in deferred_outs:
                _add_dep_helper(od.ins, last_x_dma.ins, sync=True,
                                reason="defer stores past input stream")

    nc.compile()
    return nc


def kernel(x, Ws, bs, Wt, bt, Wf, bf):
    global LAST_RESULT
    # ---- host-side weight folding (f64, weights only) ----
    A = _movavg_matrix()
    Weff = Ws.astype(np.float64) + (Wt.astype(np.float64) - Ws.astype(np.float64)) @ A
    WT = np.ascontiguousarray(Weff.T).reshape(LC, 128, P).astype(ml_dtypes.bfloat16)
    wf = Wf[0].astype(np.float64)                      # (3,)
    bias = wf.sum() * (bs.astype(np.float64) + bt.astype(np.float64)) + float(bf[0])
    bias_r = np.ascontiguousarray(bias.astype(np.float32).reshape(PC, PCW).T)

    # ---- build / compile (cached; no runtime scalars baked in) ----
    if "nc" not in _CACHE:
        _CACHE["nc"] = _build()
    nc = _CACHE["nc"]

    # ---- host-side sharding / layout; wf folded into the bf16 cast ----
    xs = x * Wf[0][None, :, None, None]                # (64,3,512,512) f32
    xb16 = xs.astype(ml_dtypes.bfloat16)
    xr = xb16.reshape(N_CORES, BPC, C, N, L)
    xr = xr.transpose(0, 2, 4, 1, 3)                   # [core, c, l, bl, n]
    xr = xr.reshape(N_CORES, C, LC, 128, BB, BNB)
    xr = xr.transpose(0, 4, 2, 3, 1, 5)                # [core, bb, lc, 128, c, bn]
    xr = xr.reshape(N_CORES, BB, LC, 128, C * BNB)

    in_maps = []
    for i in range(N_CORES):
        in_maps.append({
            "x": np.ascontiguousarray(xr[i]),
            "w": WT,
            "bias": bias_r,
        })

    res = run_bass_kernel_spmd(nc, in_maps, core_ids=list(range(N_CORES)))
    LAST_RESULT = res

    # ---- gather / unshard ----
    outs = []
    for i in range(N_CORES):
        o = res.results[i]["o"].astype(np.float32)     # (4, 112, 3*1024)
        o = o.reshape(BB, PCW, PC, BNB)
        o = o.transpose(0, 3, 2, 1).reshape(BPC, N, P)
        outs.append(o)
    out = np.stack(outs).reshape(B, N, P)[:, None]     # (64, 1, 512, 336)
    return out.astype(np.float32)
